# revision 30
# baseline (speedup 1.0000x reference)
"""Trainium2 Bass kernel for the CAM-drop attention module.

Reference computation (per sample n):
    cams  = relu(W @ x[n])            # W: [C=64, Cin=1024], x[n]: [Cin, H*W]
    thr_k = gama * max_hw(cams[k])    # per-channel spatial max
    drop  = where(cams > thr, 0, cams)
    out[n] = x[n] * mean_k(drop)      # broadcast over Cin

Data-parallel over the batch: 32 samples sharded 4-per-core across 8
NeuronCores; fc_weights / gama replicated. No cross-core communication.

HBM-bound: x is pre-cast to bf16 on the host and loaded as bf16, the
output is stored as bf16 and widened to f32 on the host (halves both HBM
streams; rel err ~9e-3, well under the 2e-2 gate). 51.4 MB of traffic per
core at the ~420 GB/s sustained per-core DMA rate bounds exec at ~125 us.

Per-core pipeline, fully overlapped across samples:
  - x streams in as per-Cin-tile DMAs on the sync HWDGE ring; weights
    load as one DMA via gpsimd so the sync ring starts on x at once.
  - PSUM is managed as 8 rotating single-bank tiles: per sample, 7 cams
    accumulator banks + 7 transient mean banks cycle through the pool, so
    the next sample's matmuls only wait on the fast per-bank relu/max
    readers instead of the whole mean-copy chain.
  - per-bank relu (ACT) and per-bank spatial max (DVE, straight from
    PSUM) chase the final accumulation matmuls; the drop-mask, mean
    matmul and PSUM->SBUF copy are interleaved per bank so the mean is
    ready a few us after the threshold.
  - out = xb * mean as whole-tile DVE muls (full tiles keep the bf16
    2x_1P perf mode; slices of larger tiles fall back to 1x). Stores
    split across the ACT HWDGE ring and the gpsimd SWDGE ring. Muls +
    stores for sample n are emitted at the top of iteration n+1 so they
    run while the next sample's matmuls accumulate.
"""

import numpy as np

# Problem shape (hardcoded per harness contract).
N, CIN, H, W = 32, 1024, 56, 56
C = 64
HW = H * W          # 3136
NCORES = 8
NS = N // NCORES    # 4 samples per core
P = 128             # SBUF partitions
NT = CIN // P       # 8 Cin tiles
NCH = 7             # spatial chunks per sample (one PSUM bank each)
CH = HW // NCH      # 448
XBUF = 18           # rotating bf16 x-tile slots (2.25 samples)
OBUF = 10           # rotating bf16 out-tile slots

_CACHE = {}


def _build_nc():
    from concourse import bacc, bass, tile
    from concourse import mybir

    f32 = mybir.dt.float32
    bf16 = mybir.dt.bfloat16
    alu = mybir.AluOpType
    RELU = mybir.ActivationFunctionType.Relu
    AX = mybir.AxisListType.X

    nc = bacc.Bacc("TRN2", target_bir_lowering=False, debug=False)
    x_ext = nc.declare_dram_parameter("x", [NS, CIN, HW], bf16, isOutput=False)
    # fc_weights pre-swizzled on host to [P, NT*C]: element (p, t*C+c) =
    # W[c, t*P+p], so w_sb[:, t*C:(t+1)*C] is the lhsT for Cin-tile t.
    w_ext = nc.declare_dram_parameter("fc_weights", [P, NT * C], bf16, isOutput=False)
    g_ext = nc.declare_dram_parameter("gama", [C, 1], f32, isOutput=False)
    out_ext = nc.declare_dram_parameter("out", [NS, CIN, HW], bf16, isOutput=True)

    with tile.TileContext(nc) as tc:
        with (
            tc.tile_pool(name="consts", bufs=1) as constp,
            tc.tile_pool(name="xbp", bufs=XBUF) as xbp,
            tc.tile_pool(name="outp", bufs=OBUF) as outp,
            tc.tile_pool(name="crelup", bufs=2 * NCH) as crelup,
            tc.tile_pool(name="meanp", bufs=2) as meanp,
            tc.tile_pool(name="statp", bufs=2) as statp,
            tc.tile_pool(name="psump", bufs=8, space=bass.MemorySpace.PSUM) as psump,
        ):
            # Consts off the sync ring (gpsimd SWDGE) so x loads start at once.
            w_sb = constp.tile([P, NT * C], bf16)
            nc.gpsimd.dma_start(out=w_sb[:], in_=w_ext[:])
            g_sb = constp.tile([C, 1], f32)
            nc.gpsimd.dma_start(out=g_sb[:], in_=g_ext[:])
            ones_sb = constp.tile([C, P], bf16)
            nc.vector.memset(ones_sb[:], 1.0 / C)

            # PE clock warm-up: the HAM gate holds the PE at half clock until
            # ~4us of sustained matmul activity. Garbage matmuls into a spare
            # rotating bank during the initial load-only DMA phase. (The 8-buf
            # rotation vs 14 tiles/sample also keeps a one-slot drift so a
            # sample's first matmul group never waits on the previous sample's
            # last mean copy.)
            warm = psump.tile([C, CH], f32, name="warm", tag="bank")
            for _ in range(15):
                nc.tensor.matmul(
                    warm[:, :], w_sb[:, 0:C], w_sb[:, 0:CH], start=True, stop=True
                )

            state = {}

            deferred = []

            def emit_muls_stores(m):
                # Stores split across the ACT HWDGE ring (t0-3) and the gpsimd
                # SWDGE ring (t4-7). For the second-to-last sample the gpsimd
                # half is deferred to the post-loop epilogue: those four tiles
                # drain during the last sample's compute chain, which would
                # otherwise leave HBM idle for ~10us — and the load stream,
                # relieved of that traffic earlier, finishes sooner.
                xbs_m, mean_m = state.pop(m)
                outs = [
                    outp.tile([P, HW], bf16, name=f"o_{m}_{t}", tag="ot")
                    for t in range(NT)
                ]
                for t in range(NT):
                    nc.vector.tensor_mul(outs[t][:], xbs_m[t][:], mean_m[:])
                    if t < 4:
                        nc.scalar.dma_start(
                            out=out_ext[m, t * P:(t + 1) * P, :], in_=outs[t][:]
                        )
                    elif m == NS - 2:
                        deferred.append((m, t, outs[t]))
                    else:
                        nc.gpsimd.dma_start(
                            out=out_ext[m, t * P:(t + 1) * P, :], in_=outs[t][:]
                        )

            for n in range(NS):
                xbs = []
                for t in range(NT):
                    xb = xbp.tile([P, HW], bf16, name=f"xb_{n}_{t}", tag="xb")
                    nc.sync.dma_start(out=xb[:], in_=x_ext[n, t * P:(t + 1) * P, :])
                    xbs.append(xb)

                # Previous sample's element-wise muls + stores run while this
                # sample's matmuls accumulate.
                if n > 0:
                    emit_muls_stores(n - 1)

                cams = [
                    psump.tile([C, CH], f32, name=f"c_{n}_{s}", tag="bank")
                    for s in range(NCH)
                ]
                for t in range(NT):
                    for s in range(NCH):
                        nc.tensor.matmul(
                            cams[s][:, :],
                            w_sb[:, t * C:(t + 1) * C],
                            xbs[t][:, s * CH:(s + 1) * CH],
                            start=(t == 0),
                            stop=(t == NT - 1),
                        )

                # Per-bank relu (ACT) chases the final accumulation matmuls
                # and is each bank's only PSUM reader, freeing it for the mean
                # matmuls and the next sample ~1us after its stop matmul.
                # crelu is 7 separate whole tiles: whole-tile DVE ops keep the
                # bf16 2x perf mode (slices of a bigger tile fall back to 1x).
                # Per-bank spatial max on DVE chases the relus; max over the
                # relu'd values * gama matches the reference threshold.
                crelus = [
                    crelup.tile([C, CH], bf16, name=f"crelu_{n}_{s}", tag="cr")
                    for s in range(NCH)
                ]
                pm = statp.tile([C, NCH], f32, name=f"pm_{n}", tag="pm")
                for s in range(NCH):
                    nc.scalar.activation(crelus[s][:, :], cams[s][:, :], RELU)
                    nc.vector.tensor_reduce(
                        pm[:, s:s + 1], crelus[s][:, :], axis=AX, op=alu.max
                    )

                # thr = gama * max(relu(cams)); bf16 thr keeps the drop-mask
                # compare all-16-bit.
                cmax = statp.tile([C, 1], f32, name=f"cmax_{n}", tag="cmax")
                nc.vector.tensor_reduce(cmax[:], pm[:, :], axis=AX, op=alu.max)
                thr = statp.tile([C, 1], bf16, name=f"thr_{n}", tag="thr")
                nc.vector.tensor_scalar(thr[:], cmax[:], g_sb[:], None, op0=alu.mult)

                # Per bank: drop-mask in place (DVE), channel mean via ones/64
                # matmul into a rotating PSUM bank (freed long ago by that
                # bank's relu/max), PSUM->SBUF copy on ACT. Interleaved so the
                # full mean trails the threshold by only a few us.
                mean_sb = meanp.tile([P, HW], bf16, name=f"mean_{n}", tag="mean")
                m3 = mean_sb[:].rearrange("p (a b) -> p a b", a=NCH)
                for s in range(NCH):
                    nc.vector.scalar_tensor_tensor(
                        crelus[s][:, :], crelus[s][:, :], thr[:],
                        crelus[s][:, :], op0=alu.is_le, op1=alu.mult,
                    )
                    mps = psump.tile([P, CH], f32, name=f"m_{n}_{s}", tag="bank")
                    nc.tensor.matmul(
                        mps[:, :], ones_sb[:], crelus[s][:, :], start=True, stop=True
                    )
                    nc.scalar.copy(m3[:, s, :], mps[:, :])

                state[n] = (xbs, mean_sb)

            for m, t, o in deferred:
                nc.gpsimd.dma_start(
                    out=out_ext[m, t * P:(t + 1) * P, :], in_=o[:]
                )
            emit_muls_stores(NS - 1)
    nc.compile()
    return nc


def _get_nc():
    if "nc" not in _CACHE:
        _CACHE["nc"] = _build_nc()
    return _CACHE["nc"]


def _make_in_maps(x, fc_weights, gama):
    from concourse import mybir

    bf16_np = mybir.dt.np(mybir.dt.bfloat16)
    x = np.asarray(x, dtype=np.float32)
    # [CIN, C] -> [NT, P, C] -> [P, NT, C] -> [P, NT*C]
    wL = np.ascontiguousarray(
        np.asarray(fc_weights, dtype=np.float32)
        .reshape(C, CIN).T
        .reshape(NT, P, C)
        .transpose(1, 0, 2)
        .reshape(P, NT * C)
    ).astype(bf16_np)
    g64 = np.ascontiguousarray(
        np.broadcast_to(np.asarray(gama, dtype=np.float32).reshape(1, 1), (C, 1))
    )
    return [
        {
            "x": np.ascontiguousarray(
                x[i * NS:(i + 1) * NS].reshape(NS, CIN, HW)
            ).astype(bf16_np),
            "fc_weights": wL,
            "gama": g64,
        }
        for i in range(NCORES)
    ]


def kernel(x: np.ndarray, fc_weights: np.ndarray, gama: np.ndarray) -> np.ndarray:
    from concourse.bass_utils import run_bass_kernel_spmd

    nc = _get_nc()
    in_maps = _make_in_maps(x, fc_weights, gama)
    res = run_bass_kernel_spmd(nc, in_maps, core_ids=list(range(NCORES)))
    out = np.concatenate(
        [
            res.results[i]["out"].astype(np.float32).reshape(NS, CIN, H, W)
            for i in range(NCORES)
        ],
        axis=0,
    )
    return out
